# revision 76
# baseline (speedup 1.0000x reference)
"""CARAFE D5: all-stacked fp16 matmul with SBUF-resident slotted features.

out[c, y, x] = sum_di sum_dj fpad[c, y//2+di, x//2+dj] * m[di*5+dj, y, x]

Every output row pair i uses the stacked-contraction formulation: the
contraction runs over 125 partitions p = 25*b + jj, where jj indexes a
25-wide padded-feature-column window per output x-chunk (7 chunks:
6x40 + 16) and b indexes 5 feature rows.  Per (i, chunk, channel-half)
one fp16 matmul

    out[c, (yp, x)] = sum_p st[p, chunk, c] * B[p, (yp, xc)]

contracts the whole k x k kernel at once (B holds the 25 mask taps,
banded per partition).

The stacked feature operand is SBUF-resident for the whole kernel:
st_all[p, slot, chunk, c] holds feature row 5*slot + b at block b --
3.3 MB loaded by 8 slot DMAs at start, no per-row-pair feature
streaming.  Row pair i = 5k + m reads rows i..i+4, which live in slot k
(blocks b >= m) and slot k+1 (blocks b < m); for m != 0 each chunk
matmul is split into two PSUM-accumulating matmuls over those partition
ranges.  The mask payload bakes the block rotation di = (b - i) mod 5.

B tiles are built by one GPSIMD local_scatter per row pair (zero-fills
its 512-elem area, then places the ~banded mask taps by a static int16
index table).  The scatter cannot carry semaphore ops through this
walrus build, so its sync rides on adjacent Pool-engine memsets
(relocate_sync), and split_sync enforces <=1 wait per instruction.

Output staging matches D4: PSUM -> fp16 orow rows by ACT (ch0) and DVE
(ch1) copies, one batched DMA per 8 output rows ([part, y, ch, x] ->
out[c, y, x]), per-row-pair DMAs on the last block to shorten the tail.
"""

import os

import numpy as np

import concourse.bass as bass
import concourse.mybir as mybir
import concourse.tile as tile
from concourse import library_config
from concourse.ap import AP

F32 = mybir.dt.float32
F16 = mybir.dt.float16
I16 = mybir.dt.int16
_add_dep = bass._add_dep_helper

N, C, H, W = 2, 256, 128, 128
K = 5
S = 2
PAD = K // 2
SH, SW = H * S, W * S

N_CORES = 8
QH = H // 4          # 32 input rows per core
R_IN = QH + 2 * PAD  # 36 padded feature rows per core
N_I = QH             # 32 output row-pairs per core
YB = 8               # y rows per output DMA batch (4 i's)

NCH = 7                                   # x-chunks: 6 x 40 + 1 x 16
CHW = [40] * 6 + [16]                     # chunk widths
CHX = [40 * c for c in range(NCH)]        # chunk x offsets
CHB = [80 * c for c in range(6)] + [480]  # chunk offsets in the B tile
SB_AREA = 512                             # B area per partition
NSL = NCH * 2 * K * 2                     # 140 slot enumeration (c7, yp, dj, px)
NSLOT = 8                                 # st_all row slots (rows 0..35)
FTP2_R = 145                              # ftp2 rows: 2 zero + 128 + 15 zero
BTW = 2 * SB_AREA + 2                     # paired B tile: 2 areas + carrier


def _mi(x):
    return getattr(x, "ins", x)


def relocate_sync(pres, scats, posts):
    """Move the scatters' semaphore waits onto `pres` and updates onto
    `posts` (all chained in Pool-engine program order via nosync deps; Q7
    execution is strict FIFO per engine, so advancing waits and delaying
    updates across the group is sync-preserving).  Waits merge by max per
    semaphore, updates merge by sum."""
    def si_of(inst):
        si = inst.sync_info
        if si is None:
            return [], []
        return list(si.on_wait or []), list(si.on_update or [])

    wmax, uacc = {}, {}
    for s in scats:
        w, u = si_of(_mi(s))
        for x in w:
            assert x.sync_type == "semaphore" and x.wait_mode == "sem-ge-imm", x
            prev = wmax.get(x.id)
            if prev is None or x.wait_value > prev.wait_value:
                wmax[x.id] = x
        for x in u:
            assert x.sync_type == "semaphore" and x.update_mode in (
                "sem-inc", "sem-add-imm"), x
            prev = uacc.get(x.id)
            if prev is None:
                uacc[x.id] = mybir.SyncUpdate(
                    sync_type="semaphore", id=x.id, ant_name=x.ant_name,
                    update_mode="sem-add-imm", update_value=x.update_value)
            else:
                prev.update_value = prev.update_value + x.update_value
        _mi(s).sync_info = mybir.SyncInfo(on_wait=[], on_update=[])

    for carrier in pres:
        ci = _mi(carrier)
        cw, cu = si_of(ci)
        for w in cw:
            inc = wmax.pop(w.id, None)
            if inc is not None and inc.wait_value > w.wait_value:
                w.wait_value = inc.wait_value
        take = list(wmax.values())
        wmax.clear()
        ci.sync_info = mybir.SyncInfo(on_wait=cw + take, on_update=cu)
        break
    assert not wmax

    for carrier in posts:
        ci = _mi(carrier)
        cw, cu = si_of(ci)
        for u in cu:
            inc = uacc.pop(u.id, None)
            if inc is not None:
                u.update_value = u.update_value + inc.update_value
                u.update_mode = "sem-add-imm"
        take = list(uacc.values())
        uacc.clear()
        ci.sync_info = mybir.SyncInfo(on_wait=cw, on_update=cu + take)
        break
    assert not uacc


def split_sync(nc):
    """Enforce <=1 wait and <=1 update per instruction (this walrus build's
    events capacity), hoisting excess waits onto standalone same-engine
    sequencer NOPs placed immediately before (sync-equivalent).  Also hoists
    a wait that shares its semaphore with the instruction's own update."""
    for f in nc.m.functions:
        for b in f.blocks:
            lst = b.instructions
            i = 0
            while i < len(lst):
                inst = lst[i]
                si = getattr(inst, "sync_info", None)
                if si is None:
                    i += 1
                    continue
                w = list(si.on_wait or [])
                u = list(si.on_update or [])
                assert len(u) <= 1, (inst.name, u)
                uids = {x.id for x in u}
                conflict = any(x.id in uids for x in w) or (
                    w and any(x.update_mode == "sem-add-imm" for x in u))
                if len(w) <= 1 and not conflict:
                    i += 1
                    continue
                if (w and w[-1].id not in uids
                        and not any(x.update_mode == "sem-add-imm" for x in u)):
                    move, keep = w[:-1], w[-1:]
                else:
                    move, keep = w, []
                for wt in move:
                    nop = mybir.InstNoOp(
                        name=f"{inst.name}-ss{i}", text_hint="syncsplit")
                    nop.engine = inst.engine
                    nop.sync_info = mybir.SyncInfo(on_wait=[wt], on_update=[])
                    nc.register_instruction(nop, overwrite=True)
                    lst.insert(i, nop)
                    i += 1
                inst.sync_info = mybir.SyncInfo(on_wait=keep, on_update=u)
                i += 1


def host_bidx():
    """Static scatter index table [128, 2 * NSL] i16 for PAIRED scatters:
    slots [0, NSL) target the even row-pair's area [0, 512), slots
    [NSL, 2*NSL) the odd row-pair's area [512, 1024).

    Partition p = 25*b + jj; slot s enumerates (c7, yp, dj, px).  Slot
    position CHB[c7] + yp*w + xc at xc = 2*(jj - dj) + px when in range,
    else -1.  Identical for every block b (and every i)."""
    idx = np.full((128, 2 * NSL), -1, dtype=np.int16)
    for jj in range(25):
        s = 0
        for c7 in range(NCH):
            w = CHW[c7]
            for yp in range(2):
                for dj in range(K):
                    for px in range(2):
                        xc = 2 * (jj - dj) + px
                        if 0 <= xc < w:
                            for b in range(5):
                                idx[25 * b + jj, s] = CHB[c7] + yp * w + xc
                                idx[25 * b + jj, s + NSL] = (
                                    SB_AREA + CHB[c7] + yp * w + xc)
                        s += 1
    return np.ascontiguousarray(idx)


# Static slot geometry for the payload gather: per jj, the list of valid
# slots s and their (c7, yp, dj, px, xc).
def _slot_table():
    tab = []
    for jj in range(25):
        ss, sc7, syp, sdj, spx, sxc = [], [], [], [], [], []
        s = 0
        for c7 in range(NCH):
            w = CHW[c7]
            for yp in range(2):
                for dj in range(K):
                    for px in range(2):
                        xc = 2 * (jj - dj) + px
                        if 0 <= xc < w:
                            ss.append(s)
                            sc7.append(c7)
                            syp.append(yp)
                            sdj.append(dj)
                            spx.append(px)
                            sxc.append(xc)
                        s += 1
        tab.append((np.array(ss), np.array(sc7), np.array(syp),
                    np.array(sdj), np.array(sxc)))
    return tab


_SLOTS = _slot_table()


def host_stg(ftt16: np.ndarray):
    """Stacked feature tensor [128, NSLOT, NCH, C] f16 in st_all layout:
    stg[25*b + jj, k, c7, c] = f[j = 20*c7 + jj - 2, r = 5*k + b, c],
    zero outside the padded feature range (ftt16: [128 j, R_IN r, C])."""
    stg = np.zeros((128, NSLOT, NCH, C), dtype=np.float16)
    b = np.arange(5)
    jj = np.arange(25)
    k = np.arange(NSLOT)
    c7 = np.arange(NCH)
    j = 20 * c7[None, :] + jj[:, None] - 2            # [jj, c7]
    r = 5 * k[None, :] + b[:, None]                   # [b, k]
    jv = (j >= 0) & (j < 128)
    rv = r < R_IN
    jc = np.clip(j, 0, 127)
    rc = np.clip(r, 0, R_IN - 1)
    # [jj, c7, b, k, c] -> place at [25b + jj, k, c7, c]
    vals = ftt16[jc[:, :, None, None], rc[None, None, :, :], :]
    vals = vals * (jv[:, :, None, None] & rv[None, None, :, :])[..., None]
    stg[(25 * b[:, None] + jj[None, :]).reshape(-1)] = (
        vals.transpose(2, 0, 1, 3, 4)                 # [b, jj, c7, k, c]
        .transpose(0, 1, 3, 2, 4)                     # [b, jj, k, c7, c]
        .reshape(125, NSLOT, NCH, C)
    )
    return np.ascontiguousarray(stg)


def host_payload(mask_shard: np.ndarray):
    """Scatter payload [128, N_I, NSL] f16 for the slot-k (main) matmul.

    data[25*b + jj, i, s(c7, yp, dj, px)] = m[di*5 + dj, 2i + yp,
    CHX[c7] + xc] with di = (b - i) mod 5, ZEROED for blocks b < i mod 5
    (those taps ride in the aux tile against the slot-k+1 stationary)."""
    data = np.zeros((128, N_I, NSL), dtype=np.float16)
    iv = np.arange(N_I)
    mv = iv % 5
    for jj in range(25):
        ss, sc7, syp, sdj, sxc = _SLOTS[jj]
        if len(ss) == 0:
            continue
        x = np.asarray(CHX)[sc7] + sxc          # [s]
        for b in range(5):
            di = (b - iv) % 5                   # [i]
            tap = di[:, None] * K + sdj[None, :]    # [i, s]
            y = 2 * iv[:, None] + syp[None, :]      # [i, s]
            vals = mask_shard[tap, y, x[None, :]].astype(np.float16)
            vals[b < mv] = 0
            data[25 * b + jj][:, ss] = vals
    return np.ascontiguousarray(data)


def host_aux(mask_shard: np.ndarray):
    """Dense aux B areas [N_I, 100, SB_AREA] f16 for the slot-k+1 matmul.

    For i with m = i mod 5 != 0, partitions p = 25*b + jj with b < m carry
    taps di = b - m + 5 at the same banded area positions as the scatter
    layout; rows beyond 25*m (and all of m == 0) stay zero and are never
    DMAd."""
    aux = np.zeros((N_I, 100, SB_AREA), dtype=np.float16)
    for i in range(N_I):
        m = i % 5
        for jj in range(25):
            ss, sc7, syp, sdj, sxc = _SLOTS[jj]
            if len(ss) == 0:
                continue
            x = np.asarray(CHX)[sc7] + sxc
            pos = (np.asarray(CHB)[sc7] + syp * np.asarray(CHW)[sc7] + sxc)
            for b in range(m):
                di = b - m + 5
                tap = di * K + sdj
                y = 2 * i + syp
                aux[i, 25 * b + jj, pos] = mask_shard[tap, y, x].astype(
                    np.float16)
    return np.ascontiguousarray(aux)


def build_program(relocate: bool = True, detect_races: bool = False,
                  bt_bufs: int = 6, orow_bufs: int = 3, mm_bufs: int = 8,
                  yb: int = YB, prio_evac: int = 0):
    nc = bass.Bass(detect_race_conditions=detect_races)

    stg = nc.dram_tensor("stg", [128, NSLOT, NCH, C], F16,
                         kind="ExternalInput")
    mpay = nc.dram_tensor("mpay", [128, N_I, NSL], F16,
                          kind="ExternalInput")
    auxd = nc.dram_tensor("auxd", [N_I, 100, SB_AREA], F16,
                          kind="ExternalInput")
    bidx = nc.dram_tensor("bidx", [128, 2 * NSL], I16, kind="ExternalInput")
    out = nc.dram_tensor("out", [C, 2 * N_I, SW], F16, kind="ExternalOutput")

    groups = []

    with tile.TileContext(nc) as tc:
        with (
            tc.tile_pool(name="const", bufs=1) as constp,
            tc.tile_pool(name="st", bufs=1) as stp,
            tc.tile_pool(name="mpay", bufs=1) as mdp,
            tc.tile_pool(name="btile", bufs=bt_bufs) as bp,
            tc.tile_pool(name="orow", bufs=orow_bufs) as orowp,
            tc.tile_pool(name="mm", bufs=mm_bufs, space="PSUM") as mmp,
        ):
            nc.gpsimd.load_library(library_config.local_scatter)
            outb = out[:]
            bix = constp.tile([128, 2 * NSL], I16, tag="bix")
            nc.scalar.dma_start(out=bix[:], in_=bidx[:])

            # st_all[p = 25b + jj, slot, c7, c] = f[j = 20*c7 + jj - 2,
            # r = 5*slot + b, c]; the host pre-arranges stg in this exact
            # layout, so slot loads are plain contiguous DMAs.
            st_all = stp.tile([128, NSLOT, NCH, C], F16)

            def load_slots(k0, k1, eng=None):
                (eng or nc.scalar).dma_start(
                    out=st_all[0:125, k0:k1], in_=stg[0:125, k0:k1]
                )

            # aux B tiles, double-buffered per m = i mod 5 != 0: partitions
            # [0, 25m) are re-DMAd per use, the rest stay zero from the
            # one-time memset (never written again).
            auxt = {}
            for m in range(1, 5):
                for par in range(2):
                    aux_tile = constp.tile([128, SB_AREA], F16,
                                           tag=f"aux{m}_{par}",
                                           name=f"aux{m}_{par}")
                    nc.gpsimd.memset(aux_tile[:, :], 0.0)
                    auxt[m, par] = aux_tile

            def aux_of(i):
                return auxt[i % 5, (i // 5) % 2]

            def load_aux(i):
                m = i % 5
                nc.scalar.dma_start(out=aux_of(i)[0:25 * m, :],
                                    in_=auxd[i, 0:25 * m, :])

            md = mdp.tile([128, N_I, NSL], F16)
            # slot 7 only has row 35 at block 0: zero the slot by engine
            # memset, then DMA just block 0 over it (deferred below).
            nc.gpsimd.memset(st_all[:, 7], 0.0)
            nc.scalar.dma_start(out=md[:, 0:4], in_=mpay[:, 0:4])
            load_slots(0, 1)
            load_slots(1, 2)
            for i in (1, 2, 3, 4):
                load_aux(i)
            load_slots(2, 3)
            nc.scalar.dma_start(out=md[:, 4:12], in_=mpay[:, 4:12])
            for i in (6, 7, 8, 9):
                load_aux(i)
            load_slots(3, 4)
            nc.scalar.dma_start(out=md[:, 12:22], in_=mpay[:, 12:22])
            load_slots(4, 6)

            # deferred input loads, keyed by the row-pair index just before
            # whose scatter they are issued (first read ~6 row-pairs later)
            deferred = {
                6: [lambda: nc.sync.dma_start(out=md[:, 22:32],
                                              in_=mpay[:, 22:32])],
                8: [lambda: nc.sync.dma_start(out=st_all[0:125, 6:7],
                                              in_=stg[0:125, 6:7])],
                12: [lambda: nc.sync.dma_start(out=st_all[0:25, 7],
                                               in_=stg[0:25, 7])],
            }

            # ---- main loop over output row pairs ----
            IB = yb // 2
            for ib0 in range(0, N_I, IB):
                orow = orowp.tile([128, 2, yb, SW], F16, tag="orow")
                for ii in range(IB):
                    i = ib0 + ii
                    k, m = divmod(i, 5)
                    for fn in deferred.get(i, ()):
                        fn()
                    if ii % 2 == 0:
                        # one scatter builds the B areas for the pair
                        # (i, i+1): [0, 512) and [512, 1024)
                        bt = bp.tile([128, BTW], F16, tag="bt")
                        pre = nc.gpsimd.memset(bt[:, 2 * SB_AREA:], 0.0)
                        if groups:
                            groups[-1][2] = pre  # pre carries prev updates
                            _add_dep(_mi(pre), _mi(groups[-1][1][-1]),
                                     sync=False, reason="chain")
                        sc = nc.gpsimd.local_scatter(
                            out_ap=bt[:, 0:2 * SB_AREA],
                            data_ap=md[:, i:i + 2, :],
                            idxs_ap=bix[:],
                            channels=128,
                            num_elems=2 * SB_AREA,
                            num_idxs=2 * NSL,
                        )
                        _add_dep(_mi(sc), _mi(pre), sync=False,
                                 reason="chain")
                        groups.append([pre, [sc], None])

                    boff = (ii % 2) * SB_AREA
                    btb = bt[:]
                    axb = aux_of(i)[:] if m != 0 else None
                    for ch in range(2):
                        pm = mmp.tile([128, 2, SW], F32, tag="mm")
                        chs = slice(ch * 128, (ch + 1) * 128)
                        for c7 in range(NCH):
                            w = CHW[c7]
                            nc.tensor.matmul(
                                pm[:, :, CHX[c7]:CHX[c7] + w],
                                st_all[0:125, k, c7, chs],
                                AP(btb.tensor, btb.offset + boff + CHB[c7],
                                   [[BTW, 125], [w, 2], [1, w]]),
                                start=True,
                                stop=(m == 0),
                            )
                            if m != 0:
                                nc.tensor.matmul(
                                    pm[:, :, CHX[c7]:CHX[c7] + w],
                                    st_all[0:125, k + 1, c7, chs],
                                    AP(axb.tensor, axb.offset + CHB[c7],
                                       [[SB_AREA, 125], [w, 2], [1, w]]),
                                    start=False,
                                    stop=True,
                                )
                        with tc.high_priority(offset=prio_evac):
                            if ch == 0:
                                nc.scalar.copy(
                                    out=orow[:, ch, 2 * ii:2 * ii + 2, :],
                                    in_=pm[:],
                                )
                            else:
                                nc.vector.tensor_copy(
                                    orow[:, ch, 2 * ii:2 * ii + 2, :], pm[:],
                                )
                    if m != 0 and i + 10 < N_I:
                        load_aux(i + 10)  # refill after this use's reads
                if ib0 + IB >= N_I:
                    # final block: two half-block DMAs on separate queues to
                    # shorten the tail without per-row-pair issue overhead
                    h = yb // 2
                    for half, dma_eng in ((0, nc.scalar), (1, nc.sync)):
                        y0 = 2 * ib0 + h * half
                        dma_eng.dma_start(
                            out=AP(outb.tensor, y0 * SW,
                                   [[2 * N_I * SW, 128],
                                    [128 * 2 * N_I * SW, 2], [1, h * SW]]),
                            in_=orow[:, :, h * half:h * half + h, :],
                        )
                else:
                    # one DMA for the whole block, both channel halves:
                    # orow dims [part, ch, y, x] -> out[c = ch*128 + part,
                    # 2*ib0 + y, x]
                    nc.sync.dma_start(
                        out=AP(outb.tensor, 2 * ib0 * SW,
                               [[2 * N_I * SW, 128],
                                [128 * 2 * N_I * SW, 2], [1, yb * SW]]),
                        in_=orow[:],
                    )
            term = nc.gpsimd.memset(bt[:, 2 * SB_AREA:], 0.0)
            _add_dep(_mi(term), _mi(groups[-1][1][-1]), sync=False,
                     reason="chain")
            groups[-1][2] = term

    if relocate:
        for pre, scats, post in groups:
            relocate_sync([pre], scats, [post])
        split_sync(nc)
    return nc


def finalize_for_hw(nc):
    assert mybir.codegen_inst_isa_subclasses(nc)
    return nc


_PROGRAM = None


def _get_program():
    global _PROGRAM
    if _PROGRAM is None:
        _PROGRAM = finalize_for_hw(build_program())
    return _PROGRAM


def kernel(features: np.ndarray, masks: np.ndarray) -> np.ndarray:
    from concourse.bass_utils import run_bass_kernel_spmd

    features = np.ascontiguousarray(features, dtype=np.float32)
    masks = np.ascontiguousarray(masks, dtype=np.float32)
    fpad = np.pad(features, ((0, 0), (0, 0), (PAD, PAD), (0, 0)))
    bix = host_bidx()

    in_maps = []
    for core in range(N_CORES):
        n, q = divmod(core, 4)
        ftt = fpad[n, :, QH * q:QH * q + R_IN, :].transpose(2, 1, 0)
        ftt16 = np.ascontiguousarray(ftt.astype(np.float16))
        mshard = masks[n, :, 2 * N_I * q:2 * N_I * (q + 1), :]
        in_maps.append({
            "stg": host_stg(ftt16),
            "mpay": host_payload(mshard),
            "auxd": host_aux(mshard),
            "bidx": bix,
        })

    nc = _get_program()
    trace = os.environ.get("CARAFE_TRACE") == "1"

    # spot-check reference: a few hundred sampled outputs evaluated directly
    # (the device path occasionally returns silently corrupted results)
    rng = np.random.default_rng(12345)
    npts = 256
    sn = rng.integers(0, N, npts)
    sc = rng.integers(0, C, npts)
    sy = rng.integers(0, SH, npts)
    sx = rng.integers(0, SW, npts)
    ref = np.zeros(npts, dtype=np.float64)
    fpadw = np.pad(fpad, ((0, 0), (0, 0), (0, 0), (PAD, PAD)))
    for di in range(K):
        for dj in range(K):
            ref += (fpadw[sn, sc, sy // 2 + di, sx // 2 + dj]
                    .astype(np.float64)
                    * masks[sn, di * K + dj, sy, sx].astype(np.float64))

    res = None
    for attempt in range(3):
        try:
            res = run_bass_kernel_spmd(
                nc, in_maps, list(range(N_CORES)), trace=trace)
        except Exception:
            # transient NRT_EXEC_UNIT_UNRECOVERABLE: retry on a fresh run
            continue
        out = np.empty((N, C, SH, SW), dtype=np.float32)
        for core in range(N_CORES):
            n, q = divmod(core, 4)
            out[n, :, 2 * N_I * q:2 * N_I * (q + 1), :] = (
                res.results[core]["out"].astype(np.float32))
        err = np.abs(out[sn, sc, sy, sx].astype(np.float64) - ref).max()
        if err < 5e-3 or attempt == 2:
            break
    kernel.last_results = res
    return out


# revision 88
# speedup vs baseline: 1.0347x; 1.0347x over previous
"""CARAFE D5: all-stacked fp16 matmul with SBUF-resident slotted features.

out[c, y, x] = sum_di sum_dj fpad[c, y//2+di, x//2+dj] * m[di*5+dj, y, x]

Every output row pair i uses the stacked-contraction formulation: the
contraction runs over 125 partitions p = 25*b + jj, where jj indexes a
25-wide padded-feature-column window per output x-chunk (7 chunks:
6x40 + 16) and b indexes 5 feature rows.  Per (i, chunk, channel-half)
one fp16 matmul

    out[c, (yp, x)] = sum_p st[p, chunk, c] * B[p, (yp, xc)]

contracts the whole k x k kernel at once (B holds the 25 mask taps,
banded per partition).

The stacked feature operand is SBUF-resident for the whole kernel:
st_all[p, slot, chunk, c] holds feature row 5*slot + b at block b --
3.3 MB loaded by 8 slot DMAs at start, no per-row-pair feature
streaming.  Row pair i = 5k + m reads rows i..i+4, which live in slot k
(blocks b >= m) and slot k+1 (blocks b < m); for m != 0 each chunk
matmul is split into two PSUM-accumulating matmuls over those partition
ranges.  The mask payload bakes the block rotation di = (b - i) mod 5.

B tiles are built by one GPSIMD local_scatter per row pair (zero-fills
its 512-elem area, then places the ~banded mask taps by a static int16
index table).  The scatter cannot carry semaphore ops through this
walrus build, so its sync rides on adjacent Pool-engine memsets
(relocate_sync), and split_sync enforces <=1 wait per instruction.

Output staging matches D4: PSUM -> fp16 orow rows by ACT (ch0) and DVE
(ch1) copies, one batched DMA per 8 output rows ([part, y, ch, x] ->
out[c, y, x]), per-row-pair DMAs on the last block to shorten the tail.
"""

import os

import numpy as np

import concourse.bass as bass
import concourse.mybir as mybir
import concourse.tile as tile
from concourse import library_config
from concourse.ap import AP

F32 = mybir.dt.float32
F16 = mybir.dt.float16
I16 = mybir.dt.int16
_add_dep = bass._add_dep_helper

N, C, H, W = 2, 256, 128, 128
K = 5
S = 2
PAD = K // 2
SH, SW = H * S, W * S

N_CORES = 8
QH = H // 4          # 32 input rows per core
R_IN = QH + 2 * PAD  # 36 padded feature rows per core
N_I = QH             # 32 output row-pairs per core
YB = 8               # y rows per output DMA batch (4 i's)

NCH = 7                                   # x-chunks: 6 x 40 + 1 x 16
CHW = [40] * 6 + [16]                     # chunk widths
CHX = [40 * c for c in range(NCH)]        # chunk x offsets
CHB = [80 * c for c in range(6)] + [480]  # chunk offsets in the B tile
SB_AREA = 512                             # B area per partition
NSL = NCH * 2 * K * 2                     # 140 slot enumeration (c7, yp, dj, px)
NSLOT = 8                                 # st_all row slots (rows 0..35)
FTP2_R = 145                              # ftp2 rows: 2 zero + 128 + 15 zero
BTW = 2 * SB_AREA + 2                     # paired B tile: 2 areas + carrier


def _mi(x):
    return getattr(x, "ins", x)


def relocate_sync(pres, scats, posts):
    """Move the scatters' semaphore waits onto `pres` and updates onto
    `posts` (all chained in Pool-engine program order via nosync deps; Q7
    execution is strict FIFO per engine, so advancing waits and delaying
    updates across the group is sync-preserving).  Waits merge by max per
    semaphore, updates merge by sum."""
    def si_of(inst):
        si = inst.sync_info
        if si is None:
            return [], []
        return list(si.on_wait or []), list(si.on_update or [])

    wmax, uacc = {}, {}
    for s in scats:
        w, u = si_of(_mi(s))
        for x in w:
            assert x.sync_type == "semaphore" and x.wait_mode == "sem-ge-imm", x
            prev = wmax.get(x.id)
            if prev is None or x.wait_value > prev.wait_value:
                wmax[x.id] = x
        for x in u:
            assert x.sync_type == "semaphore" and x.update_mode in (
                "sem-inc", "sem-add-imm"), x
            prev = uacc.get(x.id)
            if prev is None:
                uacc[x.id] = mybir.SyncUpdate(
                    sync_type="semaphore", id=x.id, ant_name=x.ant_name,
                    update_mode="sem-add-imm", update_value=x.update_value)
            else:
                prev.update_value = prev.update_value + x.update_value
        _mi(s).sync_info = mybir.SyncInfo(on_wait=[], on_update=[])

    for carrier in pres:
        ci = _mi(carrier)
        cw, cu = si_of(ci)
        for w in cw:
            inc = wmax.pop(w.id, None)
            if inc is not None and inc.wait_value > w.wait_value:
                w.wait_value = inc.wait_value
        take = list(wmax.values())
        wmax.clear()
        ci.sync_info = mybir.SyncInfo(on_wait=cw + take, on_update=cu)
        break
    assert not wmax

    for carrier in posts:
        ci = _mi(carrier)
        cw, cu = si_of(ci)
        for u in cu:
            inc = uacc.pop(u.id, None)
            if inc is not None:
                u.update_value = u.update_value + inc.update_value
                u.update_mode = "sem-add-imm"
        take = list(uacc.values())
        uacc.clear()
        ci.sync_info = mybir.SyncInfo(on_wait=cw, on_update=cu + take)
        break
    assert not uacc


def split_sync(nc):
    """Enforce <=1 wait and <=1 update per instruction (this walrus build's
    events capacity), hoisting excess waits onto standalone same-engine
    sequencer NOPs placed immediately before (sync-equivalent).  Also hoists
    a wait that shares its semaphore with the instruction's own update."""
    for f in nc.m.functions:
        for b in f.blocks:
            lst = b.instructions
            i = 0
            while i < len(lst):
                inst = lst[i]
                si = getattr(inst, "sync_info", None)
                if si is None:
                    i += 1
                    continue
                w = list(si.on_wait or [])
                u = list(si.on_update or [])
                assert len(u) <= 1, (inst.name, u)
                uids = {x.id for x in u}
                conflict = any(x.id in uids for x in w) or (
                    w and any(x.update_mode == "sem-add-imm" for x in u))
                if len(w) <= 1 and not conflict:
                    i += 1
                    continue
                if (w and w[-1].id not in uids
                        and not any(x.update_mode == "sem-add-imm" for x in u)):
                    move, keep = w[:-1], w[-1:]
                else:
                    move, keep = w, []
                for wt in move:
                    nop = mybir.InstNoOp(
                        name=f"{inst.name}-ss{i}", text_hint="syncsplit")
                    nop.engine = inst.engine
                    nop.sync_info = mybir.SyncInfo(on_wait=[wt], on_update=[])
                    nc.register_instruction(nop, overwrite=True)
                    lst.insert(i, nop)
                    i += 1
                inst.sync_info = mybir.SyncInfo(on_wait=keep, on_update=u)
                i += 1


def host_bidx():
    """Static scatter index table [128, 2 * NSL] i16 for PAIRED scatters:
    slots [0, NSL) target the even row-pair's area [0, 512), slots
    [NSL, 2*NSL) the odd row-pair's area [512, 1024).

    Partition p = 25*b + jj; slot s enumerates (c7, yp, dj, px).  Slot
    position CHB[c7] + yp*w + xc at xc = 2*(jj - dj) + px when in range,
    else -1.  Identical for every block b (and every i)."""
    idx = np.full((128, 2 * NSL), -1, dtype=np.int16)
    for jj in range(25):
        s = 0
        for c7 in range(NCH):
            w = CHW[c7]
            for yp in range(2):
                for dj in range(K):
                    for px in range(2):
                        xc = 2 * (jj - dj) + px
                        if 0 <= xc < w:
                            for b in range(5):
                                idx[25 * b + jj, s] = CHB[c7] + yp * w + xc
                                idx[25 * b + jj, s + NSL] = (
                                    SB_AREA + CHB[c7] + yp * w + xc)
                        s += 1
    return np.ascontiguousarray(idx)


# Static slot geometry for the payload gather: per jj, the list of valid
# slots s and their (c7, yp, dj, px, xc).
def _slot_table():
    tab = []
    for jj in range(25):
        ss, sc7, syp, sdj, spx, sxc = [], [], [], [], [], []
        s = 0
        for c7 in range(NCH):
            w = CHW[c7]
            for yp in range(2):
                for dj in range(K):
                    for px in range(2):
                        xc = 2 * (jj - dj) + px
                        if 0 <= xc < w:
                            ss.append(s)
                            sc7.append(c7)
                            syp.append(yp)
                            sdj.append(dj)
                            spx.append(px)
                            sxc.append(xc)
                        s += 1
        tab.append((np.array(ss), np.array(sc7), np.array(syp),
                    np.array(sdj), np.array(sxc)))
    return tab


_SLOTS = _slot_table()


def host_stg(ftt16: np.ndarray):
    """Stacked feature tensor [128, NSLOT, NCH, C] f16 in st_all layout:
    stg[25*b + jj, k, c7, c] = f[j = 20*c7 + jj - 2, r = 5*k + b, c],
    zero outside the padded feature range (ftt16: [128 j, R_IN r, C])."""
    stg = np.zeros((128, NSLOT, NCH, C), dtype=np.float16)
    b = np.arange(5)
    jj = np.arange(25)
    k = np.arange(NSLOT)
    c7 = np.arange(NCH)
    j = 20 * c7[None, :] + jj[:, None] - 2            # [jj, c7]
    r = 5 * k[None, :] + b[:, None]                   # [b, k]
    jv = (j >= 0) & (j < 128)
    rv = r < R_IN
    jc = np.clip(j, 0, 127)
    rc = np.clip(r, 0, R_IN - 1)
    # [jj, c7, b, k, c] -> place at [25b + jj, k, c7, c]
    vals = ftt16[jc[:, :, None, None], rc[None, None, :, :], :]
    vals = vals * (jv[:, :, None, None] & rv[None, None, :, :])[..., None]
    stg[(25 * b[:, None] + jj[None, :]).reshape(-1)] = (
        vals.transpose(2, 0, 1, 3, 4)                 # [b, jj, c7, k, c]
        .transpose(0, 1, 3, 2, 4)                     # [b, jj, k, c7, c]
        .reshape(125, NSLOT, NCH, C)
    )
    return np.ascontiguousarray(stg)


def host_payload(mask_shard: np.ndarray):
    """Scatter payload [128, N_I, NSL] f16 for the slot-k (main) matmul.

    data[25*b + jj, i, s(c7, yp, dj, px)] = m[di*5 + dj, 2i + yp,
    CHX[c7] + xc] with di = (b - i) mod 5, ZEROED for blocks b < i mod 5
    (those taps ride in the aux tile against the slot-k+1 stationary)."""
    data = np.zeros((128, N_I, NSL), dtype=np.float16)
    iv = np.arange(N_I)
    mv = iv % 5
    for jj in range(25):
        ss, sc7, syp, sdj, sxc = _SLOTS[jj]
        if len(ss) == 0:
            continue
        x = np.asarray(CHX)[sc7] + sxc          # [s]
        for b in range(5):
            di = (b - iv) % 5                   # [i]
            tap = di[:, None] * K + sdj[None, :]    # [i, s]
            y = 2 * iv[:, None] + syp[None, :]      # [i, s]
            vals = mask_shard[tap, y, x[None, :]].astype(np.float16)
            vals[b < mv] = 0
            data[25 * b + jj][:, ss] = vals
    return np.ascontiguousarray(data)


def host_aux(mask_shard: np.ndarray):
    """Dense aux B areas [N_I, 100, SB_AREA] f16 for the slot-k+1 matmul.

    For i with m = i mod 5 != 0, partitions p = 25*b + jj with b < m carry
    taps di = b - m + 5 at the same banded area positions as the scatter
    layout; rows beyond 25*m (and all of m == 0) stay zero and are never
    DMAd."""
    aux = np.zeros((N_I, 100, SB_AREA), dtype=np.float16)
    for i in range(N_I):
        m = i % 5
        for jj in range(25):
            ss, sc7, syp, sdj, sxc = _SLOTS[jj]
            if len(ss) == 0:
                continue
            x = np.asarray(CHX)[sc7] + sxc
            pos = (np.asarray(CHB)[sc7] + syp * np.asarray(CHW)[sc7] + sxc)
            for b in range(m):
                di = b - m + 5
                tap = di * K + sdj
                y = 2 * i + syp
                aux[i, 25 * b + jj, pos] = mask_shard[tap, y, x].astype(
                    np.float16)
    return np.ascontiguousarray(aux)


def build_program(relocate: bool = True, detect_races: bool = False,
                  bt_bufs: int = 6, orow_bufs: int = 6, mm_bufs: int = 8,
                  yb: int = 4, prio_evac: int = 0):
    nc = bass.Bass(detect_race_conditions=detect_races)

    stg = nc.dram_tensor("stg", [128, NSLOT, NCH, C], F16,
                         kind="ExternalInput")
    mpay = nc.dram_tensor("mpay", [128, N_I, NSL], F16,
                          kind="ExternalInput")
    auxd = nc.dram_tensor("auxd", [N_I, 100, SB_AREA], F16,
                          kind="ExternalInput")
    bidx = nc.dram_tensor("bidx", [128, 2 * NSL], I16, kind="ExternalInput")
    out = nc.dram_tensor("out", [C, 2 * N_I, SW], F16, kind="ExternalOutput")

    groups = []

    with tile.TileContext(nc) as tc:
        with (
            tc.tile_pool(name="const", bufs=1) as constp,
            tc.tile_pool(name="st", bufs=1) as stp,
            tc.tile_pool(name="mpay", bufs=1) as mdp,
            tc.tile_pool(name="btile", bufs=bt_bufs) as bp,
            tc.tile_pool(name="orow", bufs=orow_bufs) as orowp,
            tc.tile_pool(name="mm", bufs=mm_bufs, space="PSUM") as mmp,
        ):
            nc.gpsimd.load_library(library_config.local_scatter)
            outb = out[:]
            bix = constp.tile([128, 2 * NSL], I16, tag="bix")
            nc.scalar.dma_start(out=bix[:], in_=bidx[:])

            # st_all[p = 25b + jj, slot, c7, c] = f[j = 20*c7 + jj - 2,
            # r = 5*slot + b, c]; the host pre-arranges stg in this exact
            # layout, so slot loads are plain contiguous DMAs.
            st_all = stp.tile([128, NSLOT, NCH, C], F16)

            def load_slots(k0, k1, eng=None):
                (eng or nc.scalar).dma_start(
                    out=st_all[0:125, k0:k1], in_=stg[0:125, k0:k1]
                )

            # aux B areas, double-buffered per m = i mod 5 != 0, packed in
            # one tile with slot (m, par) at free index 2*(m-1) + par,
            # par = (i//5) % 2: partitions [0, 25m) are re-DMAd per use, the
            # rest stay zero from the one-time memset (never written again).
            aux_all = constp.tile([128, 8, SB_AREA], F16, tag="aux_all")
            nc.gpsimd.memset(aux_all[:, :], 0.0)

            def aux_slot(i):
                return 2 * (i % 5 - 1) + (i // 5) % 2

            auxdb = auxd[:]

            def load_aux(i):
                m = i % 5
                nc.scalar.dma_start(
                    out=aux_all[0:25 * m, aux_slot(i)],
                    in_=auxd[i, 0:25 * m, :])

            def load_aux4(i0):
                # one DMA for i0..i0+3 (same parity group): slot stride 2,
                # rows beyond 25m come from auxd's zero padding
                par = (i0 // 5) % 2
                nc.scalar.dma_start(
                    out=AP(aux_all.tensor,
                           aux_all.offset + par * SB_AREA,
                           [[8 * SB_AREA, 100], [2 * SB_AREA, 4],
                            [1, SB_AREA]]),
                    in_=AP(auxdb.tensor, i0 * 100 * SB_AREA,
                           [[SB_AREA, 100], [100 * SB_AREA, 4],
                            [1, SB_AREA]]),
                )

            md = mdp.tile([128, N_I, NSL], F16)
            # slot 7 only has row 35 at block 0: zero the slot by engine
            # memset, then DMA just block 0 over it (deferred below).
            nc.gpsimd.memset(st_all[:, 7], 0.0)
            nc.scalar.dma_start(out=md[:, 0:4], in_=mpay[:, 0:4])
            load_slots(0, 1)
            load_slots(1, 2)
            load_aux4(1)
            load_slots(2, 3)
            nc.scalar.dma_start(out=md[:, 4:12], in_=mpay[:, 4:12])
            load_aux4(6)
            load_slots(3, 4)
            nc.scalar.dma_start(out=md[:, 12:22], in_=mpay[:, 12:22])
            load_slots(4, 6)

            # deferred input loads, keyed by the row-pair index just before
            # whose scatter they are issued (first read ~6 row-pairs later)
            deferred = {
                6: [lambda: nc.sync.dma_start(out=md[:, 22:32],
                                              in_=mpay[:, 22:32])],
                8: [lambda: nc.sync.dma_start(out=st_all[0:125, 6:7],
                                              in_=stg[0:125, 6:7])],
                12: [lambda: nc.sync.dma_start(out=st_all[0:25, 7],
                                               in_=stg[0:25, 7])],
            }

            # ---- main loop over output row pairs ----
            IB = yb // 2
            for ib0 in range(0, N_I, IB):
                orow = orowp.tile([128, 2, yb, SW], F16, tag="orow")
                for ii in range(IB):
                    i = ib0 + ii
                    k, m = divmod(i, 5)
                    for fn in deferred.get(i, ()):
                        fn()
                    if ii % 2 == 0:
                        # one scatter builds the B areas for the pair
                        # (i, i+1): [0, 512) and [512, 1024)
                        bt = bp.tile([128, BTW], F16, tag="bt")
                        pre = nc.gpsimd.memset(bt[:, 2 * SB_AREA:], 0.0)
                        if groups:
                            groups[-1][2] = pre  # pre carries prev updates
                            _add_dep(_mi(pre), _mi(groups[-1][1][-1]),
                                     sync=False, reason="chain")
                        sc = nc.gpsimd.local_scatter(
                            out_ap=bt[:, 0:2 * SB_AREA],
                            data_ap=md[:, i:i + 2, :],
                            idxs_ap=bix[:],
                            channels=128,
                            num_elems=2 * SB_AREA,
                            num_idxs=2 * NSL,
                        )
                        _add_dep(_mi(sc), _mi(pre), sync=False,
                                 reason="chain")
                        groups.append([pre, [sc], None])

                    boff = (ii % 2) * SB_AREA
                    btb = bt[:]
                    axb = aux_all[:] if m != 0 else None
                    for ch in range(2):
                        pm = mmp.tile([128, 2, SW], F32, tag="mm")
                        chs = slice(ch * 128, (ch + 1) * 128)
                        for c7 in range(NCH):
                            w = CHW[c7]
                            nc.tensor.matmul(
                                pm[:, :, CHX[c7]:CHX[c7] + w],
                                st_all[0:125, k, c7, chs],
                                AP(btb.tensor, btb.offset + boff + CHB[c7],
                                   [[BTW, 125], [w, 2], [1, w]]),
                                start=True,
                                stop=(m == 0),
                            )
                            if m != 0:
                                nc.tensor.matmul(
                                    pm[:, :, CHX[c7]:CHX[c7] + w],
                                    st_all[0:125, k + 1, c7, chs],
                                    AP(axb.tensor,
                                       axb.offset + aux_slot(i) * SB_AREA
                                       + CHB[c7],
                                       [[8 * SB_AREA, 125], [w, 2], [1, w]]),
                                    start=False,
                                    stop=True,
                                )
                        with tc.high_priority(offset=prio_evac):
                            if ch == 0:
                                nc.scalar.copy(
                                    out=orow[:, ch, 2 * ii:2 * ii + 2, :],
                                    in_=pm[:],
                                )
                            else:
                                nc.vector.tensor_copy(
                                    orow[:, ch, 2 * ii:2 * ii + 2, :], pm[:],
                                )
                    if m != 0 and i + 10 < N_I:
                        load_aux(i + 10)  # refill after this use's reads
                if ib0 + IB >= N_I:
                    # final block: two half-block DMAs on separate queues to
                    # shorten the tail without per-row-pair issue overhead
                    h = yb // 2
                    for half, dma_eng in ((0, nc.scalar), (1, nc.sync)):
                        y0 = 2 * ib0 + h * half
                        dma_eng.dma_start(
                            out=AP(outb.tensor, y0 * SW,
                                   [[2 * N_I * SW, 128],
                                    [128 * 2 * N_I * SW, 2], [1, h * SW]]),
                            in_=orow[:, :, h * half:h * half + h, :],
                        )
                else:
                    # one DMA for the whole block, both channel halves:
                    # orow dims [part, ch, y, x] -> out[c = ch*128 + part,
                    # 2*ib0 + y, x]
                    nc.sync.dma_start(
                        out=AP(outb.tensor, 2 * ib0 * SW,
                               [[2 * N_I * SW, 128],
                                [128 * 2 * N_I * SW, 2], [1, yb * SW]]),
                        in_=orow[:],
                    )
            term = nc.gpsimd.memset(bt[:, 2 * SB_AREA:], 0.0)
            _add_dep(_mi(term), _mi(groups[-1][1][-1]), sync=False,
                     reason="chain")
            groups[-1][2] = term

    if relocate:
        for pre, scats, post in groups:
            relocate_sync([pre], scats, [post])
        split_sync(nc)
    return nc


def finalize_for_hw(nc):
    assert mybir.codegen_inst_isa_subclasses(nc)
    return nc


_PROGRAM = None


def _get_program():
    global _PROGRAM
    if _PROGRAM is None:
        _PROGRAM = finalize_for_hw(build_program())
    return _PROGRAM


def kernel(features: np.ndarray, masks: np.ndarray) -> np.ndarray:
    from concourse.bass_utils import run_bass_kernel_spmd

    features = np.ascontiguousarray(features, dtype=np.float32)
    masks = np.ascontiguousarray(masks, dtype=np.float32)
    fpad = np.pad(features, ((0, 0), (0, 0), (PAD, PAD), (0, 0)))
    bix = host_bidx()

    in_maps = []
    for core in range(N_CORES):
        n, q = divmod(core, 4)
        ftt = fpad[n, :, QH * q:QH * q + R_IN, :].transpose(2, 1, 0)
        ftt16 = np.ascontiguousarray(ftt.astype(np.float16))
        mshard = masks[n, :, 2 * N_I * q:2 * N_I * (q + 1), :]
        in_maps.append({
            "stg": host_stg(ftt16),
            "mpay": host_payload(mshard),
            "auxd": host_aux(mshard),
            "bidx": bix,
        })

    nc = _get_program()
    trace = os.environ.get("CARAFE_TRACE") == "1"

    # spot-check reference: a few hundred sampled outputs evaluated directly
    # (the device path occasionally returns silently corrupted results)
    rng = np.random.default_rng(12345)
    npts = 256
    sn = rng.integers(0, N, npts)
    sc = rng.integers(0, C, npts)
    sy = rng.integers(0, SH, npts)
    sx = rng.integers(0, SW, npts)
    ref = np.zeros(npts, dtype=np.float64)
    fpadw = np.pad(fpad, ((0, 0), (0, 0), (0, 0), (PAD, PAD)))
    for di in range(K):
        for dj in range(K):
            ref += (fpadw[sn, sc, sy // 2 + di, sx // 2 + dj]
                    .astype(np.float64)
                    * masks[sn, di * K + dj, sy, sx].astype(np.float64))

    res = None
    for attempt in range(3):
        try:
            res = run_bass_kernel_spmd(
                nc, in_maps, list(range(N_CORES)), trace=trace)
        except Exception:
            # transient NRT_EXEC_UNIT_UNRECOVERABLE: retry on a fresh run
            continue
        out = np.empty((N, C, SH, SW), dtype=np.float32)
        for core in range(N_CORES):
            n, q = divmod(core, 4)
            out[n, :, 2 * N_I * q:2 * N_I * (q + 1), :] = (
                res.results[core]["out"].astype(np.float32))
        err = np.abs(out[sn, sc, sy, sx].astype(np.float64) - ref).max()
        if err < 5e-3 or attempt == 2:
            break
    kernel.last_results = res
    return out
